# revision 1
# baseline (speedup 1.0000x reference)
"""Trainium2 Bass kernel for NoisyPQC (12-qubit noisy PQC expectation values).

Math restructure (validated vs reference in numpy):
  state index s = p*32 + f;  p = qubits 0..6 (qubit 0 = MSB of p),
  f = qubits 7..11.  state.reshape(128,32)[p,f] == state[s].
  Per trajectory r:  x = B3 D B2 D B1 D B0 psi0 with
    B0 = U0, Bl = Ul*Perm(m_{l-1})*Diag(sigma_{l-1}),
    D = (D_P (x) D_F) * C,  C[p,f] = (-1)^((p&1)*((f>>4)&1)).
  D_P/D_F fold into columns of B_l (l>=1); C applied elementwise 3x.
  Layer 0 is trajectory-independent -> host precomputes
    state1 = C * (GP0 @ psi0 @ GF0^T)  for all 16 batches.
  Device per (r): 3 layers of  phi = GP_l @ phi @ GF_l^T  (x C for l<3),
  then probs = |phi|^2, contracted with z-sign tables; final-layer noise
  becomes sign flips folded into the measurement matrices.

v2 "stationary-state" device scheme: every gate matmul uses the STATE as
the stationary (LDWEIGHTS) operand and the gate planes as the moving
operand, so out[i,j] = sum_k state[k,i]*gate[k,j] lands TRANSPOSED.
Alternating P-side / F-side multiplies then need no explicit PE
transposes at all (v1 spent 25% of PE cycles on them).

Layouts (b = 4*b_hi + b_lo):
  N: [p=128 part, col = b_hi*256 + ri*128 + b_lo*32 + f]   (ri: 0=Re,1=Im)
  T: [b_lo*32+f = 128 part, col = b_hi*256 + ri*128 + p]
P-stage (N->T), per b_hi: LDW Xre -> mm pairA=[GPr^T|GPi^T] (256 cols),
LDW Xim -> mm pairB=[-GPi^T|GPr^T] accumulate.  F-stage (T->N) same with
pairA=[kron(I4,GFr^T)|kron(I4,GFi^T)], pairB=[-kron(I4,GFi^T)|kron(I4,GFr^T)].
All 8 mms/stage stream 256 cols => full-speed f32r (1 cyc/row).

Sharding: 200 trajectories = 8 cores x 25. Each core accumulates its 25
trajectories' (sign-flipped) measurement vectors into one PSUM bank via a
single open accumulation group; host sums the 8 [24,32] outputs and /200.
"""

import sys

for _p in ("/opt/trn_rl_repo",):
    if _p not in sys.path:
        sys.path.insert(0, _p)

import numpy as np

from concourse import bacc, bass_utils, mybir
import concourse.tile as tile

# ---------------- problem constants (hardcoded per contract) ----------------
NQ = 12
NL = 4
REPS = 200
BATCH = 16
NCORES = 8
RL = REPS // NCORES  # 25 reps per core
DP, DF = 128, 32  # dim of p-side (qubits 0..6) and f-side (qubits 7..11)

F32 = mybir.dt.float32
F32R = mybir.dt.float32r


# ---------------- host-side math ----------------
def _rx(t):
    c, s = np.cos(0.5 * t), -1j * np.sin(0.5 * t)
    return np.array([[c, s], [s, c]], np.complex64)


def _rz(t):
    return np.array([[np.exp(-0.5j * t), 0], [0, np.exp(0.5j * t)]], np.complex64)


def _kron_list(mats):
    out = mats[0]
    for m in mats[1:]:
        out = np.kron(out, m)
    return out


def _tables():
    p = np.arange(DP)
    f = np.arange(DF)
    dP = np.ones(DP)
    for j in range(6):
        dP *= np.where(((p >> (6 - j)) & 1) & ((p >> (5 - j)) & 1), -1.0, 1.0)
    dF = np.ones(DF)
    for k in range(4):
        dF *= np.where(((f >> (4 - k)) & 1) & ((f >> (3 - k)) & 1), -1.0, 1.0)
    C = np.where(((p[:, None] & 1) & ((f[None, :] >> 4) & 1)) == 1, -1.0, 1.0)
    zP = 1.0 - 2.0 * ((p[None, :] >> (6 - np.arange(7)[:, None])) & 1)  # [7,128]
    zF = 1.0 - 2.0 * ((f[None, :] >> (4 - np.arange(5)[:, None])) & 1)  # [5,32]
    return dP, dF, C, zP, zF


def host_prep(data_angles, params, noise):
    """Build all device arrays. Returns (shared dict, per-core list of dicts)."""
    da = np.asarray(data_angles, np.float64)
    pa = np.asarray(params, np.float64)
    nz = np.asarray(noise)
    dPt, dFt, C, zP, zF = _tables()

    # base per-qubit gates u[l][q] = Rx(params[l,q,1]) @ Rz(params[l,q,0])
    u = [[_rx(pa[l, q, 1]) @ _rz(pa[l, q, 0]) for q in range(NQ)] for l in range(NL)]

    # --- state after layer 0 (incl. C), identical for every trajectory ---
    va = np.stack([np.cos(0.5 * da), -1j * np.sin(0.5 * da)], -1)  # [B,12,2]
    GP0 = _kron_list([u[0][q] for q in range(7)])        # [128,128]
    GF0 = _kron_list([u[0][q] for q in range(7, NQ)])    # [32,32]
    s_re = np.empty((DP, BATCH * DF), np.float32)
    s_im = np.empty((DP, BATCH * DF), np.float32)
    for b in range(BATCH):
        vPr = _kron_list([va[b, q].astype(np.complex128) for q in range(7)])
        vFr = _kron_list([va[b, q].astype(np.complex128) for q in range(7, NQ)])
        phi = (GP0 @ np.outer(vPr, vFr) @ GF0.T) * C
        s_re[:, b * DF:(b + 1) * DF] = phi.real
        s_im[:, b * DF:(b + 1) * DF] = phi.imag
    # N layout: col = b_hi*256 + ri*128 + b_lo*32 + f
    state1 = np.empty((DP, 1024), np.float32)
    for bh in range(4):
        state1[:, bh * 256:bh * 256 + 128] = s_re[:, bh * 128:bh * 128 + 128]
        state1[:, bh * 256 + 128:bh * 256 + 256] = s_im[:, bh * 128:bh * 128 + 128]

    # --- per-core moving gate tables + measurement tables ---
    eye4 = np.eye(4)
    percore = []
    for c in range(NCORES):
        gp = np.empty((RL, DP, 1536), np.float32)
        gf = np.empty((RL, DP, 1536), np.float32)
        m1 = np.empty((DP, RL * 24), np.float32)
        m2 = np.empty((DP, RL * 8), np.float32)
        for rl in range(RL):
            r = c * RL + rl
            for lidx, l in enumerate((1, 2, 3)):
                facs = []
                for q in range(NQ):
                    g = u[l][q]
                    ch = int(nz[r, l - 1, q])
                    if ch in (1, 2):
                        g = g[:, ::-1]
                    if ch in (2, 3):
                        g = g * np.array([1.0, -1.0])[None, :]
                    facs.append(g)
                GP = _kron_list(facs[:7]) * dPt[None, :]
                GF = _kron_list(facs[7:]) * dFt[None, :]
                o = lidx * 512
                gp[rl, :, o:o + 128] = GP.real.T
                gp[rl, :, o + 128:o + 256] = GP.imag.T
                gp[rl, :, o + 256:o + 384] = -GP.imag.T
                gp[rl, :, o + 384:o + 512] = GP.real.T
                A = np.kron(eye4, GF.real.T)
                Bm = np.kron(eye4, GF.imag.T)
                gf[rl, :, o:o + 128] = A
                gf[rl, :, o + 128:o + 256] = Bm
                gf[rl, :, o + 256:o + 384] = -Bm
                gf[rl, :, o + 384:o + 512] = A
            m3 = nz[r, NL - 1]  # [12]
            flip = np.where((m3 == 1) | (m3 == 2), -1.0, 1.0)
            M1 = np.ones((DF, 6))
            M1[:, :5] = (zF * flip[7:, None]).T  # [32,5]
            m1[:, rl * 24:(rl + 1) * 24] = np.kron(eye4, M1)
            M2 = np.ones((DP, 8), np.float64)
            M2[:, :7] = (zP * flip[:7, None]).T
            m2[:, rl * 8:(rl + 1) * 8] = M2
        percore.append({"gp": gp, "gf": gf, "meas1": m1, "meas2": m2})

    shared = {
        "state1": np.ascontiguousarray(state1),
        "ctabN": np.ascontiguousarray(np.tile(C, (1, 32)).astype(np.float32)),
        "ident": np.eye(DP, dtype=np.float32),
    }
    for d in percore:
        for k in list(d):
            d[k] = np.ascontiguousarray(d[k], np.float32)
    return shared, percore


def decode_output(acc):
    """acc: [24,32] summed over cores. Returns [16,12] float32."""
    out = np.empty((BATCH, NQ), np.float64)
    for bh in range(4):
        for bl in range(4):
            b = 4 * bh + bl
            for q in range(7):
                out[b, q] = acc[bl * 6 + 5, bh * 8 + q]
            for j in range(5):
                out[b, 7 + j] = acc[bl * 6 + j, bh * 8 + 7]
    return (out / REPS).astype(np.float32)


# ---------------- device kernel ----------------
def build_nc(dt=F32R, debug=False):
    """Build + compile the per-core Bass program (same for all cores)."""
    nc = bacc.Bacc("TRN2", target_bir_lowering=False, debug=debug,
                   num_devices=NCORES)
    # float32r is bit-identical to float32; declaring the DRAM side as the
    # same dtype as the SBUF tiles keeps the (hardware-DGE) DMAs cast-free.
    d_state1 = nc.dram_tensor("state1", (DP, 1024), dt, kind="ExternalInput")
    d_gp = nc.dram_tensor("gp", (RL, DP, 1536), dt, kind="ExternalInput")
    d_gf = nc.dram_tensor("gf", (RL, DP, 1536), dt, kind="ExternalInput")
    d_m1 = nc.dram_tensor("meas1", (DP, RL * 24), dt, kind="ExternalInput")
    d_m2 = nc.dram_tensor("meas2", (DP, RL * 8), dt, kind="ExternalInput")
    d_ctn = nc.dram_tensor("ctabN", (DP, 1024), F32, kind="ExternalInput")
    d_id = nc.dram_tensor("ident", (DP, DP), dt, kind="ExternalInput")
    d_out = nc.dram_tensor("out", (24, 32), F32, kind="ExternalOutput")

    MUL = mybir.AluOpType.mult
    SQ = mybir.ActivationFunctionType.Square

    with tile.TileContext(nc) as tc:
        from contextlib import ExitStack
        with ExitStack() as ex:
            cp = ex.enter_context(tc.tile_pool(name="const", bufs=1))
            sp = ex.enter_context(tc.tile_pool(name="work", bufs=1))
            pp = ex.enter_context(tc.tile_pool(name="ps", bufs=1, space="PSUM"))

            # constants -> SBUF once
            c_state1 = cp.tile([DP, 1024], dt, name="state1", tag="state1")
            c_ctn = cp.tile([DP, 1024], F32, name="ctn", tag="ctn")
            c_id = cp.tile([DP, DP], dt, name="ident", tag="ident")
            c_m1 = cp.tile([DP, RL * 24], dt, name="m1", tag="m1")
            c_m2 = cp.tile([DP, RL * 8], dt, name="m2", tag="m2")
            # consts ride on the ACT/DVE queues (their own DGE rings) so the
            # sync+gpsimd rings are free for the per-rep gate streams.
            nc.scalar.dma_start(c_state1, d_state1.ap())
            nc.scalar.dma_start(c_ctn, d_ctn.ap())
            nc.scalar.dma_start(c_m2, d_m2.ap())
            nc.scalar.dma_start(c_m1, d_m1.ap())
            nc.scalar.dma_start(c_id, d_id.ap())

            macc = pp.tile([24, 32], F32, name="macc", tag="macc")

            def cp_dve(out, in_):
                nc.vector.tensor_copy(out, in_)

            def cp_act(out, in_):
                nc.scalar.copy(out, in_)

            def emit_rep(r, ch):
                """Return list of stage closures for trajectory r on chain ch.

                Shared-slot PSUM tiles (m1p, trm) have their full write+read
                lifecycle inside a single stage, so chain interleaving never
                emits a second writer before the first reader."""
                t = {}
                g = f"{ch}"

                def s_dma():
                    # per-layer 512-col chunks on two separate rings: finer
                    # arrival granularity (first matmul starts ~2.5us in) and
                    # 2x ring bandwidth.
                    t["gp"] = sp.tile([DP, 1536], dt, name="gp", tag="gp" + g, bufs=2)
                    t["gf"] = sp.tile([DP, 1536], dt, name="gf", tag="gf" + g, bufs=2)
                    for l3 in range(3):
                        cs = slice(l3 * 512, (l3 + 1) * 512)
                        nc.sync.dma_start(t["gp"][:, cs], d_gp.ap()[r][:, cs])
                        nc.gpsimd.dma_start(t["gf"][:, cs], d_gf.ap()[r][:, cs])

                def mk_gate_mm(stat_key_or_tile, side, lidx):
                    """Stationary = state (per-b_hi 128-col blocks), moving =
                    256-col gate pair planes. Output lands layout-flipped.
                    Bank groups: bh{0,1} -> psum cols 0:512, bh{2,3} -> 512:1024."""
                    def s():
                        t["mm"] = pp.tile([DP, 1024], F32, name="mm", tag="mm" + g)
                        stat = t[stat_key_or_tile] if isinstance(stat_key_or_tile, str) \
                            else stat_key_or_tile
                        mov = t["gp"] if side == "P" else t["gf"]
                        base = lidx * 512
                        for bh in range(4):
                            o = bh * 256
                            nc.tensor.matmul(t["mm"][:, o:o + 256],
                                             stat[:, o:o + 128],
                                             mov[:, base:base + 256],
                                             start=(bh % 2 == 0), stop=False)
                            nc.tensor.matmul(t["mm"][:, o:o + 256],
                                             stat[:, o + 128:o + 256],
                                             mov[:, base + 256:base + 512],
                                             start=False, stop=(bh % 2 == 1))
                    return s

                def mk_copy(key, eng):
                    def s():
                        t[key] = sp.tile([DP, 1024], dt, name="tmp", tag="tmp" + g, bufs=2)
                        eng(t[key], t["mm"])
                    return s

                def mk_cmul(key):
                    def s():
                        t[key] = sp.tile([DP, 1024], dt, name="st", tag="st" + g, bufs=2)
                        nc.vector.tensor_tensor(t[key], t["mm"], c_ctn, MUL)
                    return s

                def s_square():
                    # write sq RI-MAJOR: col = ri*512 + bh*128 + b_lo*32 + f.
                    # Two strided ACT squares; lets the PE's own PSUM
                    # accumulation do the |re|^2+|im|^2 sum in s_meas1.
                    t["sq"] = sp.tile([DP, 1024], dt, name="sq", tag="sq" + g, bufs=2)
                    mm0 = t["mm"][:, 0:512]
                    sq0 = t["sq"][:, 0:512]
                    APc, VP = type(mm0), type(mm0.ap)
                    for ri in range(2):
                        in_ap = APc(tensor=mm0.tensor, offset=mm0.offset + ri * 128,
                                    ap=VP([[1024, DP], [256, 4], [1, 128]]))
                        out_ap = APc(tensor=sq0.tensor, offset=sq0.offset + ri * 512,
                                     ap=VP([[1024, DP], [128, 4], [1, 128]]))
                        nc.scalar.activation(out_ap, in_ap, SQ)

                def s_meas1():
                    m1p = pp.tile([8, 512], F32, name="m1p", tag="m1p" + g)
                    lhs = c_m2[:, r * 8:(r + 1) * 8]
                    nc.tensor.matmul(m1p, lhs, t["sq"][:, 0:512],
                                     start=True, stop=False)
                    nc.tensor.matmul(m1p, lhs, t["sq"][:, 512:1024],
                                     start=False, stop=True)
                    t["m1s"] = sp.tile([8, 512], dt, name="m1s", tag="m1s" + g, bufs=2)
                    nc.vector.tensor_copy(t["m1s"], m1p)

                def s_meas_tr():
                    # transpose 4 [8,128] blocks -> trm [128, (bh,q)=32]
                    trm = pp.tile([DP, 32], dt, name="trm", tag="trm")
                    for bh in range(4):
                        nc.tensor.matmul(trm[:, bh * 8:bh * 8 + 8],
                                         t["m1s"][0:8, bh * 128:(bh + 1) * 128],
                                         c_id[0:8, 0:8], is_transpose=True,
                                         start=(bh == 0), stop=(bh == 3))
                    t["trs"] = sp.tile([DP, 32], dt, name="trs", tag="trs" + g, bufs=2)
                    nc.vector.tensor_copy(t["trs"], trm)

                def s_macc():
                    nc.tensor.matmul(macc, c_m1[:, r * 24:(r + 1) * 24], t["trs"],
                                     start=(r == 0), stop=(r == RL - 1))

                return [
                    s_dma,
                    mk_gate_mm(c_state1, "P", 0),   # L1 P-side: N -> T
                    mk_copy("t1", cp_act),
                    mk_gate_mm("t1", "F", 0),       # L1 F-side: T -> N
                    mk_cmul("s1"),                  # x C (N layout)
                    mk_gate_mm("s1", "P", 1),       # L2 P-side: N -> T
                    mk_copy("t2", cp_dve),
                    mk_gate_mm("t2", "F", 1),       # L2 F-side: T -> N
                    mk_cmul("s2"),                  # x C
                    mk_gate_mm("s2", "P", 2),       # L3 P-side: N -> T
                    mk_copy("t3", cp_act),
                    mk_gate_mm("t3", "F", 2),       # L3 F-side: T -> N
                    s_square,
                    s_meas1,
                    s_meas_tr,
                    s_macc,
                ]

            # interleave two chains (even/odd trajectory) stage by stage
            r = 0
            while r < RL:
                sa = emit_rep(r, 0)
                sb = emit_rep(r + 1, 1) if r + 1 < RL else []
                for i in range(max(len(sa), len(sb))):
                    if i < len(sa):
                        sa[i]()
                    if i < len(sb):
                        sb[i]()
                r += 2

            # final: copy accumulator to SBUF, DMA out
            outs = sp.tile([24, 32], F32, name="outs", tag="outs")
            nc.vector.tensor_copy(outs, macc)
            nc.sync.dma_start(d_out.ap(), outs)

    nc.compile()
    return nc


# ---------------- public entry ----------------
_CACHE = {}


def _get_nc():
    if "nc" not in _CACHE:
        _CACHE["nc"] = build_nc()
    return _CACHE["nc"]


def run(inputs, trace=False):
    shared, percore = host_prep(inputs["data_angles"], inputs["params"],
                                inputs["noise_choices"])
    nc = _get_nc()
    in_maps = []
    for c in range(NCORES):
        m = dict(shared)
        m.update(percore[c])
        in_maps.append(m)
    res = bass_utils.run_bass_kernel_spmd(nc, in_maps, list(range(NCORES)),
                                          trace=trace)
    acc = np.zeros((24, 32), np.float64)
    for c in range(NCORES):
        acc += np.asarray(res.results[c]["out"], np.float64)
    return decode_output(acc), res


def kernel(**inputs):
    out, _ = run(inputs)
    return out



# revision 13
# speedup vs baseline: 1.2392x; 1.2392x over previous
"""Trainium2 Bass kernel for NoisyPQC (12-qubit noisy PQC expectation values).

Math restructure (validated vs reference in numpy):
  state index s = p*32 + f;  p = qubits 0..6 (qubit 0 = MSB of p),
  f = qubits 7..11.  state.reshape(128,32)[p,f] == state[s].
  Per trajectory r:  x = B3 D B2 D B1 D B0 psi0 with
    B0 = U0, Bl = Ul*Perm(m_{l-1})*Diag(sigma_{l-1}),
    D = (D_P (x) D_F) * C,  C[p,f] = (-1)^((p&1)*((f>>4)&1)).
  D_P/D_F fold into columns of B_l (l>=1); C applied elementwise 3x.
  Layer 0 is trajectory-independent -> host precomputes
    state1 = C * (GP0 @ psi0 @ GF0^T)  for all 16 batches.
  Device per (r): 3 layers of  phi = GP_l @ phi @ GF_l^T  (x C for l<3),
  then probs = |phi|^2, contracted with z-sign tables; final-layer noise
  becomes sign flips folded into the measurement matrices.

v2 "stationary-state" device scheme: every gate matmul uses the STATE as
the stationary (LDWEIGHTS) operand and the gate planes as the moving
operand, so out[i,j] = sum_k state[k,i]*gate[k,j] lands TRANSPOSED.
Alternating P-side / F-side multiplies then need no explicit PE
transposes at all (v1 spent 25% of PE cycles on them).

Layouts (b = 4*b_hi + b_lo):
  N: [p=128 part, col = b_hi*256 + ri*128 + b_lo*32 + f]   (ri: 0=Re,1=Im)
  T: [b_lo*32+f = 128 part, col = b_hi*256 + ri*128 + p]
P-stage (N->T), per b_hi: LDW Xre -> mm pairA=[GPr^T|GPi^T] (256 cols),
LDW Xim -> mm pairB=[-GPi^T|GPr^T] accumulate.  F-stage (T->N) same with
pairA=[kron(I4,GFr^T)|kron(I4,GFi^T)], pairB=[-kron(I4,GFi^T)|kron(I4,GFr^T)].
All 8 mms/stage stream 256 cols => full-speed f32r (1 cyc/row).

Sharding: 200 trajectories = 8 cores x 25. Each core accumulates its 25
trajectories' (sign-flipped) measurement vectors into one PSUM bank via a
single open accumulation group; host sums the 8 [24,32] outputs and /200.
"""

import sys

for _p in ("/opt/trn_rl_repo",):
    if _p not in sys.path:
        sys.path.insert(0, _p)

import numpy as np

from concourse import bacc, bass_utils, mybir
import concourse.tile as tile

# ---------------- problem constants (hardcoded per contract) ----------------
NQ = 12
NL = 4
REPS = 200
BATCH = 16
NCORES = 8
RL = REPS // NCORES  # 25 reps per core
DP, DF = 128, 32  # dim of p-side (qubits 0..6) and f-side (qubits 7..11)

F32 = mybir.dt.float32
F32R = mybir.dt.float32r
import ml_dtypes

BF16 = mybir.dt.bfloat16
BF16NP = ml_dtypes.bfloat16


# ---------------- host-side math ----------------
def _rx(t):
    c, s = np.cos(0.5 * t), -1j * np.sin(0.5 * t)
    return np.array([[c, s], [s, c]], np.complex64)


def _rz(t):
    return np.array([[np.exp(-0.5j * t), 0], [0, np.exp(0.5j * t)]], np.complex64)


def _kron_list(mats):
    out = mats[0]
    for m in mats[1:]:
        out = np.kron(out, m)
    return out


def _tables():
    p = np.arange(DP)
    f = np.arange(DF)
    dP = np.ones(DP)
    for j in range(6):
        dP *= np.where(((p >> (6 - j)) & 1) & ((p >> (5 - j)) & 1), -1.0, 1.0)
    dF = np.ones(DF)
    for k in range(4):
        dF *= np.where(((f >> (4 - k)) & 1) & ((f >> (3 - k)) & 1), -1.0, 1.0)
    C = np.where(((p[:, None] & 1) & ((f[None, :] >> 4) & 1)) == 1, -1.0, 1.0)
    zP = 1.0 - 2.0 * ((p[None, :] >> (6 - np.arange(7)[:, None])) & 1)  # [7,128]
    zF = 1.0 - 2.0 * ((f[None, :] >> (4 - np.arange(5)[:, None])) & 1)  # [5,32]
    return dP, dF, C, zP, zF


def host_prep(data_angles, params, noise):
    """Build all device arrays. Returns (shared dict, per-core list of dicts)."""
    da = np.asarray(data_angles, np.float64)
    pa = np.asarray(params, np.float64)
    nz = np.asarray(noise)
    dPt, dFt, C, zP, zF = _tables()

    # base per-qubit gates u[l][q] = Rx(params[l,q,1]) @ Rz(params[l,q,0])
    u = [[_rx(pa[l, q, 1]) @ _rz(pa[l, q, 0]) for q in range(NQ)] for l in range(NL)]

    # --- state after layer 0 (incl. C), identical for every trajectory ---
    va = np.stack([np.cos(0.5 * da), -1j * np.sin(0.5 * da)], -1)  # [B,12,2]
    GP0 = _kron_list([u[0][q] for q in range(7)])        # [128,128]
    GF0 = _kron_list([u[0][q] for q in range(7, NQ)])    # [32,32]
    s_re = np.empty((DP, BATCH * DF), np.float32)
    s_im = np.empty((DP, BATCH * DF), np.float32)
    for b in range(BATCH):
        vPr = _kron_list([va[b, q].astype(np.complex128) for q in range(7)])
        vFr = _kron_list([va[b, q].astype(np.complex128) for q in range(7, NQ)])
        phi = (GP0 @ np.outer(vPr, vFr) @ GF0.T) * C
        s_re[:, b * DF:(b + 1) * DF] = phi.real
        s_im[:, b * DF:(b + 1) * DF] = phi.imag
    # N layout: col = b_hi*256 + ri*128 + b_lo*32 + f
    state1 = np.empty((DP, 1024), np.float32)
    for bh in range(4):
        state1[:, bh * 256:bh * 256 + 128] = s_re[:, bh * 128:bh * 128 + 128]
        state1[:, bh * 256 + 128:bh * 256 + 256] = s_im[:, bh * 128:bh * 128 + 128]

    # --- per-core moving gate tables + measurement tables ---
    eye4 = np.eye(4)
    percore = []
    for c in range(NCORES):
        gp = np.empty((RL, DP, 1536), np.float32)
        gf = np.empty((RL, DP, 1536), np.float32)
        m1 = np.empty((DP, RL * 24), np.float32)
        m2 = np.empty((DP, RL * 8), np.float32)
        for rl in range(RL):
            r = c * RL + rl
            for lidx, l in enumerate((1, 2, 3)):
                facs = []
                for q in range(NQ):
                    g = u[l][q]
                    ch = int(nz[r, l - 1, q])
                    if ch in (1, 2):
                        g = g[:, ::-1]
                    if ch in (2, 3):
                        g = g * np.array([1.0, -1.0])[None, :]
                    facs.append(g)
                GP = _kron_list(facs[:7]) * dPt[None, :]
                GF = _kron_list(facs[7:]) * dFt[None, :]
                o = lidx * 512
                gp[rl, :, o:o + 128] = GP.real.T
                gp[rl, :, o + 128:o + 256] = GP.imag.T
                gp[rl, :, o + 256:o + 384] = -GP.imag.T
                gp[rl, :, o + 384:o + 512] = GP.real.T
                A = np.kron(eye4, GF.real.T)
                Bm = np.kron(eye4, GF.imag.T)
                gf[rl, :, o:o + 128] = A
                gf[rl, :, o + 128:o + 256] = Bm
                gf[rl, :, o + 256:o + 384] = -Bm
                gf[rl, :, o + 384:o + 512] = A
            m3 = nz[r, NL - 1]  # [12]
            flip = np.where((m3 == 1) | (m3 == 2), -1.0, 1.0)
            M1 = np.ones((DF, 6))
            M1[:, :5] = (zF * flip[7:, None]).T  # [32,5]
            m1[:, rl * 24:(rl + 1) * 24] = np.kron(eye4, M1)
            M2 = np.ones((DP, 8), np.float64)
            M2[:, :7] = (zP * flip[:7, None]).T
            m2[:, rl * 8:(rl + 1) * 8] = M2
        percore.append({"gp": gp, "gf": gf, "meas1": m1, "meas2": m2})

    shared = {
        "state1": np.ascontiguousarray(state1).astype(BF16NP),
        "ctabN": np.ascontiguousarray(np.tile(C, (1, 32)).astype(np.float32)),
    }
    for d in percore:
        for k in list(d):
            d[k] = np.ascontiguousarray(d[k].astype(BF16NP))
    return shared, percore


def decode_output(acc):
    """acc: [24,32] summed over cores. Returns [16,12] float32."""
    out = np.empty((BATCH, NQ), np.float64)
    for bh in range(4):
        for bl in range(4):
            b = 4 * bh + bl
            for q in range(7):
                out[b, q] = acc[bl * 6 + 5, bh * 8 + q]
            for j in range(5):
                out[b, 7 + j] = acc[bl * 6 + j, bh * 8 + 7]
    return (out / REPS).astype(np.float32)


# ---------------- device kernel ----------------
def build_nc(dt=BF16, debug=False):
    """Build + compile the per-core Bass program (same for all cores)."""
    nc = bacc.Bacc("TRN2", target_bir_lowering=False, debug=debug,
                   num_devices=NCORES)
    # float32r is bit-identical to float32; declaring the DRAM side as the
    # same dtype as the SBUF tiles keeps the (hardware-DGE) DMAs cast-free.
    d_state1 = nc.dram_tensor("state1", (DP, 1024), dt, kind="ExternalInput")
    d_gp = nc.dram_tensor("gp", (RL, DP, 1536), dt, kind="ExternalInput")
    d_gf = nc.dram_tensor("gf", (RL, DP, 1536), dt, kind="ExternalInput")
    d_m1 = nc.dram_tensor("meas1", (DP, RL * 24), dt, kind="ExternalInput")
    d_m2 = nc.dram_tensor("meas2", (DP, RL * 8), dt, kind="ExternalInput")
    d_ctn = nc.dram_tensor("ctabN", (DP, 1024), F32, kind="ExternalInput")
    d_out = nc.dram_tensor("out", (24, 32), F32, kind="ExternalOutput")

    MUL = mybir.AluOpType.mult
    SQ = mybir.ActivationFunctionType.Square

    with tile.TileContext(nc) as tc:
        from contextlib import ExitStack
        with ExitStack() as ex:
            cp = ex.enter_context(tc.tile_pool(name="const", bufs=1))
            sp = ex.enter_context(tc.tile_pool(name="work", bufs=1))
            pp = ex.enter_context(tc.tile_pool(name="ps", bufs=1, space="PSUM"))

            # constants -> SBUF once
            c_state1 = cp.tile([DP, 1024], dt, name="state1", tag="state1")
            c_ctn = cp.tile([DP, 1024], F32, name="ctn", tag="ctn")
            c_m1 = cp.tile([DP, RL * 24], dt, name="m1", tag="m1")
            c_m2 = cp.tile([DP, RL * 8], dt, name="m2", tag="m2")
            # consts ride on the ACT/DVE queues (their own DGE rings) so the
            # sync+gpsimd rings are free for the per-rep gate streams.
            nc.scalar.dma_start(c_state1, d_state1.ap())
            nc.scalar.dma_start(c_ctn, d_ctn.ap())
            nc.scalar.dma_start(c_m2, d_m2.ap())
            nc.scalar.dma_start(c_m1, d_m1.ap())

            macc = pp.tile([24, 32], F32, name="macc", tag="macc")

            def cp_dve(out, in_):
                nc.vector.tensor_copy(out, in_)

            def cp_act(out, in_):
                nc.scalar.copy(out, in_)

            def cp_pool(out, in_):
                nc.gpsimd.tensor_copy(out, in_)

            def emit_rep(r, ch):
                """Return list of stage closures for trajectory r on chain ch.

                Shared-slot PSUM tiles (m1p, trm) have their full write+read
                lifecycle inside a single stage, so chain interleaving never
                emits a second writer before the first reader."""
                t = {}
                g = f"{ch}"

                def s_dma():
                    # per-layer 512-col chunks on two separate rings: finer
                    # arrival granularity (first matmul starts ~2.5us in) and
                    # 2x ring bandwidth.
                    t["gp"] = sp.tile([DP, 1536], dt, name="gp", tag="gp" + g, bufs=2)
                    t["gf"] = sp.tile([DP, 1536], dt, name="gf", tag="gf" + g, bufs=2)
                    for l3 in range(3):
                        cs = slice(l3 * 512, (l3 + 1) * 512)
                        nc.sync.dma_start(t["gp"][:, cs], d_gp.ap()[r][:, cs])
                        nc.gpsimd.dma_start(t["gf"][:, cs], d_gf.ap()[r][:, cs])

                def mk_gate_mm(stat_key_or_tile, side, lidx):
                    """Stationary = state (per-b_hi 128-col blocks), moving =
                    256-col gate pair planes. Output lands layout-flipped.
                    Bank groups: bh{0,1} -> psum cols 0:512, bh{2,3} -> 512:1024."""
                    def s():
                        t["mm"] = pp.tile([DP, 1024], F32, name="mm", tag="mm" + g)
                        stat = t[stat_key_or_tile] if isinstance(stat_key_or_tile, str) \
                            else stat_key_or_tile
                        mov = t["gp"] if side == "P" else t["gf"]
                        base = lidx * 512
                        for bh in range(4):
                            o = bh * 256
                            nc.tensor.matmul(t["mm"][:, o:o + 256],
                                             stat[:, o:o + 128],
                                             mov[:, base:base + 256],
                                             start=(bh % 2 == 0), stop=False)
                            nc.tensor.matmul(t["mm"][:, o:o + 256],
                                             stat[:, o + 128:o + 256],
                                             mov[:, base + 256:base + 512],
                                             start=False, stop=(bh % 2 == 1))
                    return s

                def mk_copy(key, eng):
                    def s():
                        t[key] = sp.tile([DP, 1024], dt, name="tmp", tag="tmp" + g, bufs=2)
                        eng(t[key], t["mm"])
                    return s

                def mk_cmul(key):
                    def s():
                        t[key] = sp.tile([DP, 1024], dt, name="st", tag="st" + g, bufs=2)
                        nc.vector.tensor_tensor(t[key], t["mm"], c_ctn, MUL)
                    return s

                def s_square():
                    # write sq RI-MAJOR: col = ri*512 + bh*128 + b_lo*32 + f.
                    # Two strided ACT squares; lets the PE's own PSUM
                    # accumulation do the |re|^2+|im|^2 sum in s_meas1.
                    t["sq"] = sp.tile([DP, 1024], dt, name="sq", tag="sq" + g, bufs=2)
                    mm0 = t["mm"][:, 0:512]
                    sq0 = t["sq"][:, 0:512]
                    APc, VP = type(mm0), type(mm0.ap)
                    for ri in range(2):
                        in_ap = APc(tensor=mm0.tensor, offset=mm0.offset + ri * 128,
                                    ap=VP([[1024, DP], [256, 4], [1, 128]]))
                        out_ap = APc(tensor=sq0.tensor, offset=sq0.offset + ri * 512,
                                     ap=VP([[1024, DP], [128, 4], [1, 128]]))
                        nc.scalar.activation(out_ap, in_ap, SQ)

                def s_meas():
                    # trm[(b_lo,f), bh*8+j] = sum_p sq[p, ri*512+bh*128+c]*m2[p,j]
                    # accumulated over ri.  sq chunk is the 128-col STATIONARY,
                    # so the output lands pre-transposed -> kills the old
                    # [8,512] m1p matmuls, the m1s copy, and the PE transposes.
                    trm = pp.tile([DP, 32], F32, name="trm", tag="trm")
                    mov = c_m2[:, r * 8:(r + 1) * 8]
                    # single start/stop pair: trm is one PSUM zero-region, so
                    # only the FIRST matmul may carry start=True (a second
                    # start re-marks the whole region pending-zero and wipes
                    # the already-written neighbour columns).
                    for ri in range(2):
                        for bh in range(4):
                            chunk = t["sq"][:, ri * 512 + bh * 128:
                                            ri * 512 + (bh + 1) * 128]
                            nc.tensor.matmul(trm[:, bh * 8:(bh + 1) * 8],
                                             chunk, mov,
                                             start=(ri == 0 and bh == 0),
                                             stop=(ri == 1 and bh == 3),
                                             skip_group_check=True)
                    t["trs"] = sp.tile([DP, 32], dt, name="trs", tag="trs" + g, bufs=2)
                    nc.vector.tensor_copy(t["trs"], trm)

                def s_macc():
                    nc.tensor.matmul(macc, c_m1[:, r * 24:(r + 1) * 24], t["trs"],
                                     start=(r == 0), stop=(r == RL - 1))

                return [
                    s_dma,
                    mk_gate_mm(c_state1, "P", 0),   # L1 P-side: N -> T
                    mk_copy("t1", cp_act),
                    mk_gate_mm("t1", "F", 0),       # L1 F-side: T -> N
                    mk_cmul("s1"),                  # x C (N layout)
                    mk_gate_mm("s1", "P", 1),       # L2 P-side: N -> T
                    mk_copy("t2", cp_dve),
                    mk_gate_mm("t2", "F", 1),       # L2 F-side: T -> N
                    mk_cmul("s2"),                  # x C
                    mk_gate_mm("s2", "P", 2),       # L3 P-side: N -> T
                    mk_copy("t3", cp_act),
                    mk_gate_mm("t3", "F", 2),       # L3 F-side: T -> N
                    s_square,
                    s_meas,
                    s_macc,
                ]

            # interleave three chains stage by stage: covers the PSUM->SBUF
            # evacuation latency so the PE never drains (and stays un-throttled)
            NCH = 3
            r = 0
            while r < RL:
                chains = [emit_rep(r + j, j) for j in range(NCH) if r + j < RL]
                for i in range(max(len(s) for s in chains)):
                    for s in chains:
                        if i < len(s):
                            s[i]()
                r += NCH

            # final: copy accumulator to SBUF, DMA out
            outs = sp.tile([24, 32], F32, name="outs", tag="outs")
            nc.vector.tensor_copy(outs, macc)
            nc.sync.dma_start(d_out.ap(), outs)

    nc.compile()
    return nc


# ---------------- public entry ----------------
_CACHE = {}


def _get_nc():
    if "nc" not in _CACHE:
        _CACHE["nc"] = build_nc()
    return _CACHE["nc"]


def run(inputs, trace=False):
    shared, percore = host_prep(inputs["data_angles"], inputs["params"],
                                inputs["noise_choices"])
    nc = _get_nc()
    in_maps = []
    for c in range(NCORES):
        m = dict(shared)
        m.update(percore[c])
        in_maps.append(m)
    res = bass_utils.run_bass_kernel_spmd(nc, in_maps, list(range(NCORES)),
                                          trace=trace)
    acc = np.zeros((24, 32), np.float64)
    for c in range(NCORES):
        acc += np.asarray(res.results[c]["out"], np.float64)
    return decode_output(acc), res


def kernel(**inputs):
    out, _ = run(inputs)
    return out



# revision 43
# speedup vs baseline: 1.3465x; 1.0866x over previous
"""Trainium2 Bass kernel for NoisyPQC (12-qubit noisy PQC expectation values).

Math restructure (validated vs reference in numpy):
  state index s = p*32 + f;  p = qubits 0..6 (qubit 0 = MSB of p),
  f = qubits 7..11.  state.reshape(128,32)[p,f] == state[s].
  Per trajectory r:  x = B3 D B2 D B1 D B0 psi0 with
    B0 = U0, Bl = Ul*Perm(m_{l-1})*Diag(sigma_{l-1}),
    D = (D_P (x) D_F) * C,  C[p,f] = (-1)^((p&1)*((f>>4)&1)).
  D_P/D_F fold into columns of B_l (l>=1); C applied elementwise 3x.
  Layer 0 is trajectory-independent -> host precomputes
    state1 = C * (GP0 @ psi0 @ GF0^T)  for all 16 batches.
  Device per (r): 3 layers of  phi = GP_l @ phi @ GF_l^T  (x C for l<3),
  then probs = |phi|^2, contracted with z-sign tables; final-layer noise
  becomes sign flips folded into the measurement matrices.

v2 "stationary-state" device scheme: every gate matmul uses the STATE as
the stationary (LDWEIGHTS) operand and the gate planes as the moving
operand, so out[i,j] = sum_k state[k,i]*gate[k,j] lands TRANSPOSED.
Alternating P-side / F-side multiplies then need no explicit PE
transposes at all (v1 spent 25% of PE cycles on them).

Layouts (b = 4*b_hi + b_lo):
  N: [p=128 part, col = b_hi*256 + ri*128 + b_lo*32 + f]   (ri: 0=Re,1=Im)
  T: [b_lo*32+f = 128 part, col = b_hi*256 + ri*128 + p]
P-stage (N->T), per b_hi: LDW Xre -> mm pairA=[GPr^T|GPi^T] (256 cols),
LDW Xim -> mm pairB=[-GPi^T|GPr^T] accumulate.  F-stage (T->N) same with
pairA=[kron(I4,GFr^T)|kron(I4,GFi^T)], pairB=[-kron(I4,GFi^T)|kron(I4,GFr^T)].
All 8 mms/stage stream 256 cols => full-speed f32r (1 cyc/row).

Sharding: 200 trajectories = 8 cores x 25. Each core accumulates its 25
trajectories' (sign-flipped) measurement vectors into one PSUM bank via a
single open accumulation group; host sums the 8 [24,32] outputs and /200.
"""

import sys

for _p in ("/opt/trn_rl_repo",):
    if _p not in sys.path:
        sys.path.insert(0, _p)

import numpy as np

from concourse import bacc, bass_utils, mybir
import concourse.tile as tile

# ---------------- problem constants (hardcoded per contract) ----------------
NQ = 12
NL = 4
REPS = 200
BATCH = 16
NCORES = 8
RL = REPS // NCORES  # 25 reps per core
DP, DF = 128, 32  # dim of p-side (qubits 0..6) and f-side (qubits 7..11)

F32 = mybir.dt.float32
F32R = mybir.dt.float32r
import ml_dtypes

BF16 = mybir.dt.bfloat16
BF16NP = ml_dtypes.bfloat16
FP8 = mybir.dt.float8e4
FP8NP = ml_dtypes.float8_e4m3fn

# fp8-e4m3 DoubleRow path: gate/state tensors in fp8, one K=256 matmul per
# complex product.  State scaled by SSCALE so amplitudes sit in e4m3's
# normal range (RMS elem ~1/90 -> ~0.7 after scaling); squares/meas in bf16.
USE_DR = False
SSCALE = 64.0
# (bf16 PSUM matmul outputs are rejected by hardware: "matmul output must
# be fp32" -- PSUM stays f32, 2 banks per in-flight gate stage.)
PSUM_BF16 = False


# ---------------- host-side math ----------------
def _rx(t):
    c, s = np.cos(0.5 * t), -1j * np.sin(0.5 * t)
    return np.array([[c, s], [s, c]], np.complex64)


def _rz(t):
    return np.array([[np.exp(-0.5j * t), 0], [0, np.exp(0.5j * t)]], np.complex64)


def _kron_list(mats):
    out = mats[0]
    for m in mats[1:]:
        out = np.kron(out, m)
    return out


def _tables():
    p = np.arange(DP)
    f = np.arange(DF)
    dP = np.ones(DP)
    for j in range(6):
        dP *= np.where(((p >> (6 - j)) & 1) & ((p >> (5 - j)) & 1), -1.0, 1.0)
    dF = np.ones(DF)
    for k in range(4):
        dF *= np.where(((f >> (4 - k)) & 1) & ((f >> (3 - k)) & 1), -1.0, 1.0)
    C = np.where(((p[:, None] & 1) & ((f[None, :] >> 4) & 1)) == 1, -1.0, 1.0)
    zP = 1.0 - 2.0 * ((p[None, :] >> (6 - np.arange(7)[:, None])) & 1)  # [7,128]
    zF = 1.0 - 2.0 * ((f[None, :] >> (4 - np.arange(5)[:, None])) & 1)  # [5,32]
    return dP, dF, C, zP, zF


def host_prep(data_angles, params, noise):
    """Build all device arrays. Returns (shared dict, per-core list of dicts)."""
    da = np.asarray(data_angles, np.float64)
    pa = np.asarray(params, np.float64)
    nz = np.asarray(noise)
    dPt, dFt, C, zP, zF = _tables()

    # base per-qubit gates u[l][q] = Rx(params[l,q,1]) @ Rz(params[l,q,0])
    u = [[_rx(pa[l, q, 1]) @ _rz(pa[l, q, 0]) for q in range(NQ)] for l in range(NL)]

    # --- state after layer 0 (incl. C), identical for every trajectory ---
    va = np.stack([np.cos(0.5 * da), -1j * np.sin(0.5 * da)], -1)  # [B,12,2]
    GP0 = _kron_list([u[0][q] for q in range(7)])        # [128,128]
    GF0 = _kron_list([u[0][q] for q in range(7, NQ)])    # [32,32]
    s_re = np.empty((DP, BATCH * DF), np.float32)
    s_im = np.empty((DP, BATCH * DF), np.float32)
    for b in range(BATCH):
        vPr = _kron_list([va[b, q].astype(np.complex128) for q in range(7)])
        vFr = _kron_list([va[b, q].astype(np.complex128) for q in range(7, NQ)])
        phi = (GP0 @ np.outer(vPr, vFr) @ GF0.T) * C
        s_re[:, b * DF:(b + 1) * DF] = phi.real
        s_im[:, b * DF:(b + 1) * DF] = phi.imag
    # N layout: col = b_hi*256 + ri*128 + b_lo*32 + f
    state1 = np.empty((DP, 1024), np.float32)
    for bh in range(4):
        state1[:, bh * 256:bh * 256 + 128] = s_re[:, bh * 128:bh * 128 + 128]
        state1[:, bh * 256 + 128:bh * 256 + 256] = s_im[:, bh * 128:bh * 128 + 128]

    # --- per-core moving gate tables + measurement tables ---
    eye4 = np.eye(4)
    percore = []
    for c in range(NCORES):
        gp = np.empty((RL, DP, 1536), np.float32)
        gf = np.empty((RL, DP, 1536), np.float32)
        m1 = np.empty((DP, RL * 24), np.float32)
        m2 = np.empty((DP, RL * 8), np.float32)
        for rl in range(RL):
            r = c * RL + rl
            for lidx, l in enumerate((1, 2, 3)):
                facs = []
                for q in range(NQ):
                    g = u[l][q]
                    ch = int(nz[r, l - 1, q])
                    if ch in (1, 2):
                        g = g[:, ::-1]
                    if ch in (2, 3):
                        g = g * np.array([1.0, -1.0])[None, :]
                    facs.append(g)
                GP = _kron_list(facs[:7]) * dPt[None, :]
                GF = _kron_list(facs[7:]) * dFt[None, :]
                o = lidx * 512
                gp[rl, :, o:o + 128] = GP.real.T
                gp[rl, :, o + 128:o + 256] = GP.imag.T
                gp[rl, :, o + 256:o + 384] = -GP.imag.T
                gp[rl, :, o + 384:o + 512] = GP.real.T
                A = np.kron(eye4, GF.real.T)
                Bm = np.kron(eye4, GF.imag.T)
                gf[rl, :, o:o + 128] = A
                gf[rl, :, o + 128:o + 256] = Bm
                gf[rl, :, o + 256:o + 384] = -Bm
                gf[rl, :, o + 384:o + 512] = A
            m3 = nz[r, NL - 1]  # [12]
            flip = np.where((m3 == 1) | (m3 == 2), -1.0, 1.0)
            M1 = np.ones((DF, 6))
            M1[:, :5] = (zF * flip[7:, None]).T  # [32,5]
            m1[:, rl * 24:(rl + 1) * 24] = np.kron(eye4, M1)
            M2 = np.ones((DP, 8), np.float64)
            M2[:, :7] = (zP * flip[:7, None]).T
            m2[:, rl * 8:(rl + 1) * 8] = M2
        percore.append({"gp": gp, "gf": gf, "meas1": m1, "meas2": m2})

    shared = {
        "ctabN": np.ascontiguousarray(np.tile(C, (1, 32))).astype(BF16NP),
    }
    if not USE_DR:
        shared["state1"] = np.ascontiguousarray(state1).astype(BF16NP)
        for d in percore:
            for k in list(d):
                d[k] = np.ascontiguousarray(d[k].astype(BF16NP))
        return shared, percore

    # fp8 path.  state1: A+B residual split (A=fp8(x), B=fp8(x-A)) so the
    # rep-shared initial state keeps ~2x mantissa; layout per bh:
    # col = bh*512 + resid*256 + ri*128 + c.
    s1 = state1 * SSCALE
    sA = s1.astype(FP8NP)
    sB = (s1 - sA.astype(np.float64)).astype(FP8NP)
    s2x = np.empty((DP, 2048), FP8NP)
    for bh in range(4):
        s2x[:, bh * 512:bh * 512 + 256] = sA[:, bh * 256:(bh + 1) * 256]
        s2x[:, bh * 512 + 256:bh * 512 + 512] = sB[:, bh * 256:(bh + 1) * 256]
    shared["state1"] = np.ascontiguousarray(s2x)

    # gate tables: per-rep dithered fp8 quantization (zero-mean across reps,
    # so the quantization error averages out of the trajectory mean).
    rng = np.random.default_rng(12345)
    for d in percore:
        for k in list(d):
            if k in ("gp", "gf"):
                x = d[k].astype(np.float64)
                q = np.where(x == 0.0, 0.0,
                             2.0 ** np.maximum(
                                 np.floor(np.log2(np.abs(x) + 1e-300)) - 3, -9))
                x = x + (rng.random(x.shape) - 0.5) * q
                d[k] = np.ascontiguousarray(x.astype(FP8NP))
            else:
                d[k] = np.ascontiguousarray(d[k].astype(BF16NP))
    return shared, percore


def decode_output(acc):
    """acc: [24,32] summed over cores. Returns [16,12] float32."""
    out = np.empty((BATCH, NQ), np.float64)
    for bh in range(4):
        for bl in range(4):
            b = 4 * bh + bl
            for q in range(7):
                out[b, q] = acc[bl * 6 + 5, bh * 8 + q]
            for j in range(5):
                out[b, 7 + j] = acc[bl * 6 + j, bh * 8 + 7]
    scale = REPS * (SSCALE * SSCALE if USE_DR else 1.0)
    return (out / scale).astype(np.float32)


# ---------------- device kernel ----------------
def build_nc(dt=BF16, debug=False):
    """Build + compile the per-core Bass program (same for all cores)."""
    nc = bacc.Bacc("TRN2", target_bir_lowering=False, debug=debug,
                   num_devices=NCORES)
    gdt = FP8 if USE_DR else dt  # gate/state dtype
    s1w = 2048 if USE_DR else 1024  # state1 width (A+B residual split in fp8)
    d_state1 = nc.dram_tensor("state1", (DP, s1w), gdt, kind="ExternalInput")
    d_gp = nc.dram_tensor("gp", (RL, DP, 1536), gdt, kind="ExternalInput")
    d_gf = nc.dram_tensor("gf", (RL, DP, 1536), gdt, kind="ExternalInput")
    d_m1 = nc.dram_tensor("meas1", (DP, RL * 24), dt, kind="ExternalInput")
    d_m2 = nc.dram_tensor("meas2", (DP, RL * 8), dt, kind="ExternalInput")
    d_ctn = nc.dram_tensor("ctabN", (DP, 1024), BF16, kind="ExternalInput")
    d_out = nc.dram_tensor("out", (24, 32), F32, kind="ExternalOutput")

    MUL = mybir.AluOpType.mult
    SQ = mybir.ActivationFunctionType.Square

    with tile.TileContext(nc) as tc:
        from contextlib import ExitStack
        with ExitStack() as ex:
            cp = ex.enter_context(tc.tile_pool(name="const", bufs=1))
            sp = ex.enter_context(tc.tile_pool(name="work", bufs=1))
            pp = ex.enter_context(tc.tile_pool(name="ps", bufs=1, space="PSUM"))

            # constants -> SBUF once
            c_state1 = cp.tile([DP, s1w], gdt, name="state1", tag="state1")
            c_ctn = cp.tile([DP, 1024], BF16, name="ctn", tag="ctn")
            c_m1 = cp.tile([DP, RL * 24], dt, name="m1", tag="m1")
            c_m2 = cp.tile([DP, RL * 8], dt, name="m2", tag="m2")
            # consts ride on the ACT/DVE queues (their own DGE rings) so the
            # sync+gpsimd rings are free for the per-rep gate streams.
            # state1 feeds the very first matmul: split across two rings
            # (gpsimd's first gf chunk isn't needed until after L1-P anyway)
            h = s1w // 2
            nc.scalar.dma_start(c_state1[:, 0:h], d_state1.ap()[:, 0:h])
            nc.gpsimd.dma_start(c_state1[:, h:s1w], d_state1.ap()[:, h:s1w])
            nc.scalar.dma_start(c_ctn, d_ctn.ap())
            nc.scalar.dma_start(c_m2, d_m2.ap())
            nc.scalar.dma_start(c_m1, d_m1.ap())

            macc = pp.tile([24, 32], F32, name="macc", tag="macc")

            def cp_dve(out, in_):
                nc.vector.tensor_copy(out, in_)

            def cp_act(out, in_):
                nc.scalar.copy(out, in_)

            def cp_pool(out, in_):
                nc.gpsimd.tensor_copy(out, in_)

            def emit_rep(r, ch):
                """Return list of stage closures for trajectory r on chain ch.

                Shared-slot PSUM tiles (m1p, trm) have their full write+read
                lifecycle inside a single stage, so chain interleaving never
                emits a second writer before the first reader."""
                t = {}
                g = f"{ch}"

                def s_dma():
                    # per-layer 512-col chunks on two separate rings: finer
                    # arrival granularity (first matmul starts ~2.5us in) and
                    # 2x ring bandwidth.
                    t["gp"] = sp.tile([DP, 1536], gdt, name="gp", tag="gp" + g, bufs=2)
                    t["gf"] = sp.tile([DP, 1536], gdt, name="gf", tag="gf" + g, bufs=2)
                    for l3 in range(3):
                        cs = slice(l3 * 512, (l3 + 1) * 512)
                        nc.sync.dma_start(t["gp"][:, cs], d_gp.ap()[r][:, cs])
                        nc.gpsimd.dma_start(t["gf"][:, cs], d_gf.ap()[r][:, cs])

                def mk_gate_mm(stat_key_or_tile, side, lidx):
                    """Stationary = state (per-b_hi 128-col blocks), moving =
                    256-col gate pair planes. Output lands layout-flipped.
                    Bank groups: bh{0,1} -> psum cols 0:512, bh{2,3} -> 512:1024.
                    USE_DR: one fp8 DoubleRow matmul per bh does the whole
                    complex product (K=256: lhsT pairs = re/im state blocks,
                    rhs pairs = pairA/pairB gate planes)."""
                    def s():
                        t["mm"] = pp.tile([DP, 1024], BF16 if PSUM_BF16 else F32,
                                          name="mm", tag="mm" + g)
                        stat = t[stat_key_or_tile] if isinstance(stat_key_or_tile, str) \
                            else stat_key_or_tile
                        mov = t["gp"] if side == "P" else t["gf"]
                        base = lidx * 512
                        if USE_DR:
                            s0, m0 = stat[:, 0:1], mov[:, 0:1]
                            APc, VP = type(s0), type(s0.ap)
                            is_s1 = stat is c_state1
                            sw = s1w if is_s1 else 1024
                            nres = 2 if is_s1 else 1  # state1 A+B residual
                            for bh in range(4):
                                o = bh * 256
                                rhs = APc(tensor=m0.tensor, offset=m0.offset + base,
                                          ap=VP([[1536, DP], [256, 2], [1, 256]]))
                                for j in range(nres):
                                    so = bh * 512 + j * 256 if is_s1 else o
                                    lhsT = APc(tensor=s0.tensor,
                                               offset=s0.offset + so,
                                               ap=VP([[sw, DP], [128, 2], [1, 128]]))
                                    nc.tensor.matmul(
                                        t["mm"][:, o:o + 256], lhsT, rhs,
                                        start=(bh % 2 == 0 and j == 0),
                                        stop=(bh % 2 == 1 and j == nres - 1),
                                        perf_mode=mybir.MatmulPerfMode.DoubleRow)
                            return
                        for bh in range(4):
                            o = bh * 256
                            # bf16 psum: whole tile is ONE 2KB zero region ->
                            # exactly one start and one stop for the stage.
                            first = (bh == 0) if PSUM_BF16 else (bh % 2 == 0)
                            last = (bh == 3) if PSUM_BF16 else (bh % 2 == 1)
                            nc.tensor.matmul(t["mm"][:, o:o + 256],
                                             stat[:, o:o + 128],
                                             mov[:, base:base + 256],
                                             start=first, stop=False)
                            nc.tensor.matmul(t["mm"][:, o:o + 256],
                                             stat[:, o + 128:o + 256],
                                             mov[:, base + 256:base + 512],
                                             start=False, stop=last)
                    return s

                def mk_copy(key, eng):
                    def s():
                        t[key] = sp.tile([DP, 1024], gdt, name="tmp", tag="tmp" + g, bufs=2)
                        eng(t[key], t["mm"])
                    return s

                def mk_cmul(key):
                    def s():
                        t[key] = sp.tile([DP, 1024], gdt, name="st", tag="st" + g, bufs=2)
                        nc.vector.tensor_tensor(t[key], t["mm"], c_ctn, MUL)
                    return s

                def s_square():
                    # |phi|^2 in the natural N layout (col = bh*256+ri*128+c);
                    # s_meas picks its (ri, bh) chunks straight out of it.
                    t["sq"] = sp.tile([DP, 1024], dt, name="sq", tag="sq" + g, bufs=2)
                    nc.scalar.activation(t["sq"], t["mm"], SQ)

                def s_meas():
                    # trm[(b_lo,f), bh*8+j] = sum_p sq[p, bh*256+ri*128+c]*m2[p,j]
                    # accumulated over ri.  sq chunk is the 128-col STATIONARY,
                    # so the output lands pre-transposed -> kills the old
                    # [8,512] m1p matmuls, the m1s copy, and the PE transposes.
                    trm = pp.tile([DP, 32], F32, name="trm", tag="trm")
                    mov = c_m2[:, r * 8:(r + 1) * 8]
                    # single start/stop pair: trm is one PSUM zero-region, so
                    # only the FIRST matmul may carry start=True (a second
                    # start re-marks the whole region pending-zero and wipes
                    # the already-written neighbour columns).
                    for ri in range(2):
                        for bh in range(4):
                            chunk = t["sq"][:, bh * 256 + ri * 128:
                                            bh * 256 + (ri + 1) * 128]
                            nc.tensor.matmul(trm[:, bh * 8:(bh + 1) * 8],
                                             chunk, mov,
                                             start=(ri == 0 and bh == 0),
                                             stop=(ri == 1 and bh == 3),
                                             skip_group_check=True)
                    t["trs"] = sp.tile([DP, 32], dt, name="trs", tag="trs" + g, bufs=2)
                    # ACT, not DVE: releases the shared trm slot without
                    # queueing behind the big DVE evacuations.
                    nc.scalar.copy(t["trs"], trm)

                def s_macc():
                    nc.tensor.matmul(macc, c_m1[:, r * 24:(r + 1) * 24], t["trs"],
                                     start=(r == 0), stop=(r == RL - 1))

                return [
                    s_dma,
                    mk_gate_mm(c_state1, "P", 0),   # L1 P-side: N -> T
                    mk_copy("t1", cp_act),
                    mk_gate_mm("t1", "F", 0),       # L1 F-side: T -> N
                    mk_cmul("s1"),                  # x C (N layout)
                    mk_gate_mm("s1", "P", 1),       # L2 P-side: N -> T
                    mk_copy("t2", cp_dve),
                    mk_gate_mm("t2", "F", 1),       # L2 F-side: T -> N
                    mk_cmul("s2"),                  # x C
                    mk_gate_mm("s2", "P", 2),       # L3 P-side: N -> T
                    mk_copy("t3", cp_act),
                    mk_gate_mm("t3", "F", 2),       # L3 F-side: T -> N
                    s_square,
                    s_meas,
                    s_macc,
                ]

            # Skewed software pipeline: rep r's stage i is emitted at tick
            # r*OFF + i, so ~16/OFF reps are in flight at STAGGERED stages.
            # (In-phase chains all hit the shared trm slot / the same engine
            # at once and stall the PE at every round boundary.)
            NCH = 3   # tag sets (PSUM: NCH * 2 mm banks + trm + macc <= 8)
            OFF = 5
            all_stages = [emit_rep(r, r % NCH) for r in range(RL)]
            events = []
            for r in range(RL):
                for i in range(len(all_stages[r])):
                    events.append((r * OFF + i, r, i))
            events.sort(key=lambda e: (e[0], e[1]))
            for _, r, i in events:
                all_stages[r][i]()

            # final: copy accumulator to SBUF, DMA out
            outs = sp.tile([24, 32], F32, name="outs", tag="outs")
            nc.vector.tensor_copy(outs, macc)
            nc.sync.dma_start(d_out.ap(), outs)

    nc.compile()
    return nc


# ---------------- public entry ----------------
_CACHE = {}


def _get_nc():
    if "nc" not in _CACHE:
        _CACHE["nc"] = build_nc()
    return _CACHE["nc"]


def run(inputs, trace=False):
    shared, percore = host_prep(inputs["data_angles"], inputs["params"],
                                inputs["noise_choices"])
    nc = _get_nc()
    in_maps = []
    for c in range(NCORES):
        m = dict(shared)
        m.update(percore[c])
        in_maps.append(m)
    res = bass_utils.run_bass_kernel_spmd(nc, in_maps, list(range(NCORES)),
                                          trace=trace)
    acc = np.zeros((24, 32), np.float64)
    for c in range(NCORES):
        acc += np.asarray(res.results[c]["out"], np.float64)
    return decode_output(acc), res


def kernel(**inputs):
    out, _ = run(inputs)
    return out



# revision 44
# speedup vs baseline: 1.3942x; 1.0354x over previous
"""Trainium2 Bass kernel for NoisyPQC (12-qubit noisy PQC expectation values).

Math restructure (validated vs reference in numpy):
  state index s = p*32 + f;  p = qubits 0..6 (qubit 0 = MSB of p),
  f = qubits 7..11.  state.reshape(128,32)[p,f] == state[s].
  Per trajectory r:  x = B3 D B2 D B1 D B0 psi0 with
    B0 = U0, Bl = Ul*Perm(m_{l-1})*Diag(sigma_{l-1}),
    D = (D_P (x) D_F) * C,  C[p,f] = (-1)^((p&1)*((f>>4)&1)).
  D_P/D_F fold into columns of B_l (l>=1); C applied elementwise 3x.
  Layer 0 is trajectory-independent -> host precomputes
    state1 = C * (GP0 @ psi0 @ GF0^T)  for all 16 batches.
  Device per (r): 3 layers of  phi = GP_l @ phi @ GF_l^T  (x C for l<3),
  then probs = |phi|^2, contracted with z-sign tables; final-layer noise
  becomes sign flips folded into the measurement matrices.

v2 "stationary-state" device scheme: every gate matmul uses the STATE as
the stationary (LDWEIGHTS) operand and the gate planes as the moving
operand, so out[i,j] = sum_k state[k,i]*gate[k,j] lands TRANSPOSED.
Alternating P-side / F-side multiplies then need no explicit PE
transposes at all (v1 spent 25% of PE cycles on them).

Layouts (b = 4*b_hi + b_lo):
  N: [p=128 part, col = b_hi*256 + ri*128 + b_lo*32 + f]   (ri: 0=Re,1=Im)
  T: [b_lo*32+f = 128 part, col = b_hi*256 + ri*128 + p]
P-stage (N->T), per b_hi: LDW Xre -> mm pairA=[GPr^T|GPi^T] (256 cols),
LDW Xim -> mm pairB=[-GPi^T|GPr^T] accumulate.  F-stage (T->N) same with
pairA=[kron(I4,GFr^T)|kron(I4,GFi^T)], pairB=[-kron(I4,GFi^T)|kron(I4,GFr^T)].
All 8 mms/stage stream 256 cols => full-speed f32r (1 cyc/row).

Sharding: 200 trajectories = 8 cores x 25. Each core accumulates its 25
trajectories' (sign-flipped) measurement vectors into one PSUM bank via a
single open accumulation group; host sums the 8 [24,32] outputs and /200.
"""

import sys

for _p in ("/opt/trn_rl_repo",):
    if _p not in sys.path:
        sys.path.insert(0, _p)

import numpy as np

from concourse import bacc, bass_utils, mybir
import concourse.tile as tile

# ---------------- problem constants (hardcoded per contract) ----------------
NQ = 12
NL = 4
REPS = 200
BATCH = 16
NCORES = 8
RL = REPS // NCORES  # 25 reps per core
DP, DF = 128, 32  # dim of p-side (qubits 0..6) and f-side (qubits 7..11)

F32 = mybir.dt.float32
F32R = mybir.dt.float32r
import ml_dtypes

BF16 = mybir.dt.bfloat16
BF16NP = ml_dtypes.bfloat16
FP8 = mybir.dt.float8e4
FP8NP = ml_dtypes.float8_e4m3fn

# fp8-e4m3 DoubleRow path: gate/state tensors in fp8, one K=256 matmul per
# complex product.  State scaled by SSCALE so amplitudes sit in e4m3's
# normal range (RMS elem ~1/90 -> ~0.7 after scaling); squares/meas in bf16.
USE_DR = False
SSCALE = 64.0
# (bf16 PSUM matmul outputs are rejected by hardware: "matmul output must
# be fp32" -- PSUM stays f32, 2 banks per in-flight gate stage.)
PSUM_BF16 = False


# ---------------- host-side math ----------------
def _rx(t):
    c, s = np.cos(0.5 * t), -1j * np.sin(0.5 * t)
    return np.array([[c, s], [s, c]], np.complex64)


def _rz(t):
    return np.array([[np.exp(-0.5j * t), 0], [0, np.exp(0.5j * t)]], np.complex64)


def _kron_list(mats):
    out = mats[0]
    for m in mats[1:]:
        out = np.kron(out, m)
    return out


def _tables():
    p = np.arange(DP)
    f = np.arange(DF)
    dP = np.ones(DP)
    for j in range(6):
        dP *= np.where(((p >> (6 - j)) & 1) & ((p >> (5 - j)) & 1), -1.0, 1.0)
    dF = np.ones(DF)
    for k in range(4):
        dF *= np.where(((f >> (4 - k)) & 1) & ((f >> (3 - k)) & 1), -1.0, 1.0)
    C = np.where(((p[:, None] & 1) & ((f[None, :] >> 4) & 1)) == 1, -1.0, 1.0)
    zP = 1.0 - 2.0 * ((p[None, :] >> (6 - np.arange(7)[:, None])) & 1)  # [7,128]
    zF = 1.0 - 2.0 * ((f[None, :] >> (4 - np.arange(5)[:, None])) & 1)  # [5,32]
    return dP, dF, C, zP, zF


def host_prep(data_angles, params, noise):
    """Build all device arrays. Returns (shared dict, per-core list of dicts)."""
    da = np.asarray(data_angles, np.float64)
    pa = np.asarray(params, np.float64)
    nz = np.asarray(noise)
    dPt, dFt, C, zP, zF = _tables()

    # base per-qubit gates u[l][q] = Rx(params[l,q,1]) @ Rz(params[l,q,0])
    u = [[_rx(pa[l, q, 1]) @ _rz(pa[l, q, 0]) for q in range(NQ)] for l in range(NL)]

    # --- state after layer 0 (incl. C), identical for every trajectory ---
    va = np.stack([np.cos(0.5 * da), -1j * np.sin(0.5 * da)], -1)  # [B,12,2]
    GP0 = _kron_list([u[0][q] for q in range(7)])        # [128,128]
    GF0 = _kron_list([u[0][q] for q in range(7, NQ)])    # [32,32]
    s_re = np.empty((DP, BATCH * DF), np.float32)
    s_im = np.empty((DP, BATCH * DF), np.float32)
    for b in range(BATCH):
        vPr = _kron_list([va[b, q].astype(np.complex128) for q in range(7)])
        vFr = _kron_list([va[b, q].astype(np.complex128) for q in range(7, NQ)])
        phi = (GP0 @ np.outer(vPr, vFr) @ GF0.T) * C
        s_re[:, b * DF:(b + 1) * DF] = phi.real
        s_im[:, b * DF:(b + 1) * DF] = phi.imag
    # N layout: col = b_hi*256 + ri*128 + b_lo*32 + f
    state1 = np.empty((DP, 1024), np.float32)
    for bh in range(4):
        state1[:, bh * 256:bh * 256 + 128] = s_re[:, bh * 128:bh * 128 + 128]
        state1[:, bh * 256 + 128:bh * 256 + 256] = s_im[:, bh * 128:bh * 128 + 128]

    # --- per-core moving gate tables + measurement tables ---
    eye4 = np.eye(4)
    percore = []
    for c in range(NCORES):
        gp = np.empty((RL, DP, 1536), np.float32)
        gf = np.empty((RL, DP, 1536), np.float32)
        m1 = np.empty((DP, RL * 24), np.float32)
        m2 = np.empty((DP, RL * 8), np.float32)
        for rl in range(RL):
            r = c * RL + rl
            for lidx, l in enumerate((1, 2, 3)):
                facs = []
                for q in range(NQ):
                    g = u[l][q]
                    ch = int(nz[r, l - 1, q])
                    if ch in (1, 2):
                        g = g[:, ::-1]
                    if ch in (2, 3):
                        g = g * np.array([1.0, -1.0])[None, :]
                    facs.append(g)
                GP = _kron_list(facs[:7]) * dPt[None, :]
                GF = _kron_list(facs[7:]) * dFt[None, :]
                o = lidx * 512
                gp[rl, :, o:o + 128] = GP.real.T
                gp[rl, :, o + 128:o + 256] = GP.imag.T
                gp[rl, :, o + 256:o + 384] = -GP.imag.T
                gp[rl, :, o + 384:o + 512] = GP.real.T
                A = np.kron(eye4, GF.real.T)
                Bm = np.kron(eye4, GF.imag.T)
                gf[rl, :, o:o + 128] = A
                gf[rl, :, o + 128:o + 256] = Bm
                gf[rl, :, o + 256:o + 384] = -Bm
                gf[rl, :, o + 384:o + 512] = A
            m3 = nz[r, NL - 1]  # [12]
            flip = np.where((m3 == 1) | (m3 == 2), -1.0, 1.0)
            M1 = np.ones((DF, 6))
            M1[:, :5] = (zF * flip[7:, None]).T  # [32,5]
            m1[:, rl * 24:(rl + 1) * 24] = np.kron(eye4, M1)
            M2 = np.ones((DP, 8), np.float64)
            M2[:, :7] = (zP * flip[:7, None]).T
            m2[:, rl * 8:(rl + 1) * 8] = M2
        percore.append({"gp": gp, "gf": gf, "meas1": m1, "meas2": m2})

    shared = {
        "ctabN": np.ascontiguousarray(np.tile(C, (1, 32))).astype(BF16NP),
    }
    if not USE_DR:
        shared["state1"] = np.ascontiguousarray(state1).astype(BF16NP)
        for d in percore:
            for k in list(d):
                d[k] = np.ascontiguousarray(d[k].astype(BF16NP))
        return shared, percore

    # fp8 path.  state1: A+B residual split (A=fp8(x), B=fp8(x-A)) so the
    # rep-shared initial state keeps ~2x mantissa; layout per bh:
    # col = bh*512 + resid*256 + ri*128 + c.
    s1 = state1 * SSCALE
    sA = s1.astype(FP8NP)
    sB = (s1 - sA.astype(np.float64)).astype(FP8NP)
    s2x = np.empty((DP, 2048), FP8NP)
    for bh in range(4):
        s2x[:, bh * 512:bh * 512 + 256] = sA[:, bh * 256:(bh + 1) * 256]
        s2x[:, bh * 512 + 256:bh * 512 + 512] = sB[:, bh * 256:(bh + 1) * 256]
    shared["state1"] = np.ascontiguousarray(s2x)

    # gate tables: per-rep dithered fp8 quantization (zero-mean across reps,
    # so the quantization error averages out of the trajectory mean).
    rng = np.random.default_rng(12345)
    for d in percore:
        for k in list(d):
            if k in ("gp", "gf"):
                x = d[k].astype(np.float64)
                q = np.where(x == 0.0, 0.0,
                             2.0 ** np.maximum(
                                 np.floor(np.log2(np.abs(x) + 1e-300)) - 3, -9))
                x = x + (rng.random(x.shape) - 0.5) * q
                d[k] = np.ascontiguousarray(x.astype(FP8NP))
            else:
                d[k] = np.ascontiguousarray(d[k].astype(BF16NP))
    return shared, percore


def decode_output(acc):
    """acc: [24,32] summed over cores. Returns [16,12] float32."""
    out = np.empty((BATCH, NQ), np.float64)
    for bh in range(4):
        for bl in range(4):
            b = 4 * bh + bl
            for q in range(7):
                out[b, q] = acc[bl * 6 + 5, bh * 8 + q]
            for j in range(5):
                out[b, 7 + j] = acc[bl * 6 + j, bh * 8 + 7]
    scale = REPS * (SSCALE * SSCALE if USE_DR else 1.0)
    return (out / scale).astype(np.float32)


# ---------------- device kernel ----------------
def build_nc(dt=BF16, debug=False):
    """Build + compile the per-core Bass program (same for all cores)."""
    nc = bacc.Bacc("TRN2", target_bir_lowering=False, debug=debug,
                   num_devices=NCORES)
    gdt = FP8 if USE_DR else dt  # gate/state dtype
    s1w = 2048 if USE_DR else 1024  # state1 width (A+B residual split in fp8)
    d_state1 = nc.dram_tensor("state1", (DP, s1w), gdt, kind="ExternalInput")
    d_gp = nc.dram_tensor("gp", (RL, DP, 1536), gdt, kind="ExternalInput")
    d_gf = nc.dram_tensor("gf", (RL, DP, 1536), gdt, kind="ExternalInput")
    d_m1 = nc.dram_tensor("meas1", (DP, RL * 24), dt, kind="ExternalInput")
    d_m2 = nc.dram_tensor("meas2", (DP, RL * 8), dt, kind="ExternalInput")
    d_ctn = nc.dram_tensor("ctabN", (DP, 1024), BF16, kind="ExternalInput")
    d_out = nc.dram_tensor("out", (24, 32), F32, kind="ExternalOutput")

    MUL = mybir.AluOpType.mult
    SQ = mybir.ActivationFunctionType.Square

    with tile.TileContext(nc) as tc:
        from contextlib import ExitStack
        with ExitStack() as ex:
            cp = ex.enter_context(tc.tile_pool(name="const", bufs=1))
            sp = ex.enter_context(tc.tile_pool(name="work", bufs=1))
            pp = ex.enter_context(tc.tile_pool(name="ps", bufs=1, space="PSUM"))

            # constants -> SBUF once
            c_state1 = cp.tile([DP, s1w], gdt, name="state1", tag="state1")
            c_ctn = cp.tile([DP, 1024], BF16, name="ctn", tag="ctn")
            c_m1 = cp.tile([DP, RL * 24], dt, name="m1", tag="m1")
            c_m2 = cp.tile([DP, RL * 8], dt, name="m2", tag="m2")
            # consts ride on the ACT/DVE queues (their own DGE rings) so the
            # sync+gpsimd rings are free for the per-rep gate streams.
            # state1 feeds the very first matmul: split across two rings
            # (gpsimd's first gf chunk isn't needed until after L1-P anyway)
            h = s1w // 2
            nc.scalar.dma_start(c_state1[:, 0:h], d_state1.ap()[:, 0:h])
            nc.gpsimd.dma_start(c_state1[:, h:s1w], d_state1.ap()[:, h:s1w])
            nc.scalar.dma_start(c_ctn, d_ctn.ap())
            nc.scalar.dma_start(c_m2, d_m2.ap())
            nc.scalar.dma_start(c_m1, d_m1.ap())

            macc = pp.tile([24, 32], F32, name="macc", tag="macc")

            def cp_dve(out, in_):
                nc.vector.tensor_copy(out, in_)

            def cp_act(out, in_):
                nc.scalar.copy(out, in_)

            def cp_pool(out, in_):
                nc.gpsimd.tensor_copy(out, in_)

            def emit_rep(r, ch):
                """Return list of stage closures for trajectory r on chain ch.

                Shared-slot PSUM tiles (m1p, trm) have their full write+read
                lifecycle inside a single stage, so chain interleaving never
                emits a second writer before the first reader."""
                t = {}
                g = f"{ch}"

                def s_dma():
                    # per-layer 512-col chunks on two separate rings: finer
                    # arrival granularity (first matmul starts ~2.5us in) and
                    # 2x ring bandwidth.
                    t["gp"] = sp.tile([DP, 1536], gdt, name="gp", tag="gp" + g, bufs=2)
                    t["gf"] = sp.tile([DP, 1536], gdt, name="gf", tag="gf" + g, bufs=2)
                    for l3 in range(3):
                        cs = slice(l3 * 512, (l3 + 1) * 512)
                        nc.sync.dma_start(t["gp"][:, cs], d_gp.ap()[r][:, cs])
                        nc.gpsimd.dma_start(t["gf"][:, cs], d_gf.ap()[r][:, cs])

                def mk_gate_mm(stat_key_or_tile, side, lidx):
                    """Stationary = state (per-b_hi 128-col blocks), moving =
                    256-col gate pair planes. Output lands layout-flipped.
                    Bank groups: bh{0,1} -> psum cols 0:512, bh{2,3} -> 512:1024.
                    USE_DR: one fp8 DoubleRow matmul per bh does the whole
                    complex product (K=256: lhsT pairs = re/im state blocks,
                    rhs pairs = pairA/pairB gate planes)."""
                    def s():
                        t["mm"] = pp.tile([DP, 1024], BF16 if PSUM_BF16 else F32,
                                          name="mm", tag="mm" + g)
                        stat = t[stat_key_or_tile] if isinstance(stat_key_or_tile, str) \
                            else stat_key_or_tile
                        mov = t["gp"] if side == "P" else t["gf"]
                        base = lidx * 512
                        if USE_DR:
                            s0, m0 = stat[:, 0:1], mov[:, 0:1]
                            APc, VP = type(s0), type(s0.ap)
                            is_s1 = stat is c_state1
                            sw = s1w if is_s1 else 1024
                            nres = 2 if is_s1 else 1  # state1 A+B residual
                            for bh in range(4):
                                o = bh * 256
                                rhs = APc(tensor=m0.tensor, offset=m0.offset + base,
                                          ap=VP([[1536, DP], [256, 2], [1, 256]]))
                                for j in range(nres):
                                    so = bh * 512 + j * 256 if is_s1 else o
                                    lhsT = APc(tensor=s0.tensor,
                                               offset=s0.offset + so,
                                               ap=VP([[sw, DP], [128, 2], [1, 128]]))
                                    nc.tensor.matmul(
                                        t["mm"][:, o:o + 256], lhsT, rhs,
                                        start=(bh % 2 == 0 and j == 0),
                                        stop=(bh % 2 == 1 and j == nres - 1),
                                        perf_mode=mybir.MatmulPerfMode.DoubleRow)
                            return
                        for bh in range(4):
                            o = bh * 256
                            # bf16 psum: whole tile is ONE 2KB zero region ->
                            # exactly one start and one stop for the stage.
                            first = (bh == 0) if PSUM_BF16 else (bh % 2 == 0)
                            last = (bh == 3) if PSUM_BF16 else (bh % 2 == 1)
                            nc.tensor.matmul(t["mm"][:, o:o + 256],
                                             stat[:, o:o + 128],
                                             mov[:, base:base + 256],
                                             start=first, stop=False)
                            nc.tensor.matmul(t["mm"][:, o:o + 256],
                                             stat[:, o + 128:o + 256],
                                             mov[:, base + 256:base + 512],
                                             start=False, stop=last)
                    return s

                def mk_copy(key, eng):
                    def s():
                        t[key] = sp.tile([DP, 1024], gdt, name="tmp", tag="tmp" + g, bufs=2)
                        eng(t[key], t["mm"])
                    return s

                def mk_cmul(key):
                    def s():
                        t[key] = sp.tile([DP, 1024], gdt, name="st", tag="st" + g, bufs=2)
                        nc.vector.tensor_tensor(t[key], t["mm"], c_ctn, MUL)
                    return s

                def s_square():
                    # |phi|^2 in the natural N layout (col = bh*256+ri*128+c);
                    # s_meas picks its (ri, bh) chunks straight out of it.
                    t["sq"] = sp.tile([DP, 1024], dt, name="sq", tag="sq" + g, bufs=2)
                    nc.scalar.activation(t["sq"], t["mm"], SQ)

                def s_meas():
                    # trm[(b_lo,f), bh*8+j] = sum_p sq[p, bh*256+ri*128+c]*m2[p,j]
                    # accumulated over ri.  sq chunk is the 128-col STATIONARY,
                    # so the output lands pre-transposed -> kills the old
                    # [8,512] m1p matmuls, the m1s copy, and the PE transposes.
                    trm = pp.tile([DP, 32], F32, name="trm", tag="trm")
                    mov = c_m2[:, r * 8:(r + 1) * 8]
                    # single start/stop pair: trm is one PSUM zero-region, so
                    # only the FIRST matmul may carry start=True (a second
                    # start re-marks the whole region pending-zero and wipes
                    # the already-written neighbour columns).
                    for ri in range(2):
                        for bh in range(4):
                            chunk = t["sq"][:, bh * 256 + ri * 128:
                                            bh * 256 + (ri + 1) * 128]
                            nc.tensor.matmul(trm[:, bh * 8:(bh + 1) * 8],
                                             chunk, mov,
                                             start=(ri == 0 and bh == 0),
                                             stop=(ri == 1 and bh == 3),
                                             skip_group_check=True)
                    t["trs"] = sp.tile([DP, 32], dt, name="trs", tag="trs" + g, bufs=2)
                    # ACT, not DVE: releases the shared trm slot without
                    # queueing behind the big DVE evacuations.
                    nc.scalar.copy(t["trs"], trm)

                def s_macc():
                    nc.tensor.matmul(macc, c_m1[:, r * 24:(r + 1) * 24], t["trs"],
                                     start=(r == 0), stop=(r == RL - 1))

                return [
                    s_dma,
                    mk_gate_mm(c_state1, "P", 0),   # L1 P-side: N -> T
                    mk_copy("t1", cp_act),
                    mk_gate_mm("t1", "F", 0),       # L1 F-side: T -> N
                    mk_cmul("s1"),                  # x C (N layout)
                    mk_gate_mm("s1", "P", 1),       # L2 P-side: N -> T
                    mk_copy("t2", cp_dve),
                    mk_gate_mm("t2", "F", 1),       # L2 F-side: T -> N
                    mk_cmul("s2"),                  # x C
                    mk_gate_mm("s2", "P", 2),       # L3 P-side: N -> T
                    mk_copy("t3", cp_act),
                    mk_gate_mm("t3", "F", 2),       # L3 F-side: T -> N
                    s_square,
                    s_meas,
                    s_macc,
                ]

            # Skewed software pipeline: rep r's stage i is emitted at tick
            # r*OFF + i, so ~16/OFF reps are in flight at STAGGERED stages.
            # (In-phase chains all hit the shared trm slot / the same engine
            # at once and stall the PE at every round boundary.)
            NCH = 3   # tag sets (PSUM: NCH * 2 mm banks + trm + macc <= 8)
            OFF = 4
            all_stages = [emit_rep(r, r % NCH) for r in range(RL)]
            events = []
            for r in range(RL):
                for i in range(len(all_stages[r])):
                    events.append((r * OFF + i, r, i))
            events.sort(key=lambda e: (e[0], e[1]))
            for _, r, i in events:
                all_stages[r][i]()

            # final: copy accumulator to SBUF, DMA out
            outs = sp.tile([24, 32], F32, name="outs", tag="outs")
            nc.vector.tensor_copy(outs, macc)
            nc.sync.dma_start(d_out.ap(), outs)

    nc.compile()
    return nc


# ---------------- public entry ----------------
_CACHE = {}


def _get_nc():
    if "nc" not in _CACHE:
        _CACHE["nc"] = build_nc()
    return _CACHE["nc"]


def run(inputs, trace=False):
    shared, percore = host_prep(inputs["data_angles"], inputs["params"],
                                inputs["noise_choices"])
    nc = _get_nc()
    in_maps = []
    for c in range(NCORES):
        m = dict(shared)
        m.update(percore[c])
        in_maps.append(m)
    res = bass_utils.run_bass_kernel_spmd(nc, in_maps, list(range(NCORES)),
                                          trace=trace)
    acc = np.zeros((24, 32), np.float64)
    for c in range(NCORES):
        acc += np.asarray(res.results[c]["out"], np.float64)
    return decode_output(acc), res


def kernel(**inputs):
    out, _ = run(inputs)
    return out

